# revision 3
# baseline (speedup 1.0000x reference)
"""Vocab-parallel dense layer (x @ mat^T + bias) on 8 TRN2 NeuronCores.

Full-input contract: kernel(x, mat, bias) takes the unsharded numpy inputs
  x    (4096, 1, 1024) f32
  mat  (1, 32000, 1024) f32
  bias (1, 32000) f32
and returns the full (4096, 32000) f32 output.

Sharding: mat/bias are split over num_classes into 8 shards of 4000 columns
(tensor/vocab parallel); x is replicated.  Each core computes its
(4096, 4000) output slice; the host concatenates.

Device kernel (per core), structured so the PE never waits:
  - Host lays out x as 32 contiguous per-batch-block tiles xm[m] =
    [128p, k*128+b] and mat as 8 contiguous per-class-slice tiles
    mj[j] = [128p, k*500+c], both bf16.
  - Input DMAs are sequenced on the two HWDGE FIFO rings so tiles land in
    consumption order: SP ring carries x m-blocks (256 KB each, first one
    lands ~1.5us in), ACT ring carries bias then mat j-slices (1 MB each).
    Output DMAs are emitted on the SP ring behind all x loads, so stores
    cannot steal HBM bandwidth from the load phase; by steady state the
    ring drains faster than evictions are produced.
  - Loop j (8 class slices) -> m (32 batch tiles) -> k (8 K-tiles): each
    (j,m) is one 8-matmul accumulation k-run into a single PSUM bank,
    banks rotating 8-deep, so a bank's eviction overlaps the next 7
    k-runs instead of clustering at the end of a batch tile.  The
    stationary x slice changes every matmul (per-MM LDWEIGHTS); on HW the
    PE's 64-deep reorder window + dual SBUF read ports hide the weight
    loads behind the 500-column streams.
  - PSUM eviction fuses the bias add (vector.tensor_add) and feeds the
    output DMA, buffered 24 deep so stores can lag the load phase.
  - Matmuls are chained with ordering-only deps to keep the PE stream in
    emission order.
"""

import numpy as np

import concourse.bass as bass
import concourse.bacc as bacc
import concourse.tile as tile
from concourse import mybir
from concourse.bass import _add_dep_helper
from concourse.bass_utils import run_bass_kernel_spmd

# Problem geometry (hardcoded; harness runs this file standalone).
B = 4096          # batch
E = 1024          # embed size (contraction dim K)
C = 32000         # num classes
NCORES = 8
CS = C // NCORES  # classes per core (4000)

P = 128           # SBUF partitions / matmul tile K and M
KT = E // P       # 8 K-tiles
MT = B // P       # 32 batch tiles
NTILE = 500       # moving free dim per matmul (<=512, one PSUM bank in f32)
NT = CS // NTILE  # 8 class tiles per batch tile
OUTBUF = 24       # SBUF out-tile ring depth (absorbs store lag in load phase)

_BF16 = mybir.dt.np(mybir.dt.bfloat16)

_program_cache = {}


def _build_program(iters=1):
    bf16, f32 = mybir.dt.bfloat16, mybir.dt.float32

    nc = bacc.Bacc("TRN2", target_bir_lowering=False, debug=False,
                   num_devices=NCORES)
    xm = nc.dram_tensor("xm", (MT, P, KT * P), bf16,
                        kind="ExternalInput").ap()
    mj = nc.dram_tensor("mj", (NT, P, KT * NTILE), bf16,
                        kind="ExternalInput").ap()
    bias = nc.dram_tensor("bias", (1, CS), f32, kind="ExternalInput").ap()
    out = nc.dram_tensor("out", (B, CS), f32, kind="ExternalOutput").ap()

    with tile.TileContext(nc) as tc:
        with tc.tile_pool(name="resident", bufs=1) as resident, \
             tc.tile_pool(name="psum", bufs=8, space="PSUM") as psums, \
             tc.tile_pool(name="outs", bufs=OUTBUF) as outs:

            def body():
                # ACT ring: mat slice j0 first (gates the first matmul),
                # then the bias broadcast (first needed by the first
                # eviction, ~16us in), then the remaining mat slices in
                # consumption order.
                bias_sb = resident.tile([P, CS], f32, tag="bias",
                                        name="bias_sb")
                msb = []
                for j in range(NT):
                    mt = resident.tile([P, KT * NTILE], bf16, tag=f"m{j}",
                                       name=f"m{j}")
                    nc.scalar.dma_start(out=mt[:], in_=mj[j])
                    msb.append(mt)
                    if j == 0:
                        nc.scalar.dma_start(out=bias_sb[:],
                                            in_=bias.to_broadcast((P, CS)))
                # SP ring: x blocks in consumption order; output stores are
                # emitted later on this same FIFO ring so they queue behind
                # every x load.
                xsb = []
                for m in range(MT):
                    xt = resident.tile([P, KT * P], bf16, tag=f"x{m}",
                                       name=f"x{m}")
                    nc.sync.dma_start(out=xt[:], in_=xm[m])
                    xsb.append(xt)

                prev_mm = None
                for j in range(NT):
                    for m in range(MT):
                        ps = psums.tile([P, NTILE], f32, tag="ps",
                                        name=f"ps{j}_{m}")
                        for k in range(KT):
                            mm = nc.tensor.matmul(
                                ps[:],
                                xsb[m][:, k * P:(k + 1) * P],
                                msb[j][:, k * NTILE:(k + 1) * NTILE],
                                start=(k == 0), stop=(k == KT - 1))
                            # ordering-only chain keeps the PE stream in
                            # emission order
                            if prev_mm is not None:
                                _add_dep_helper(mm.ins, prev_mm.ins,
                                                sync=False,
                                                reason="pe-order")
                            prev_mm = mm
                        ot = outs.tile([P, NTILE], f32, tag="ot",
                                       name=f"ot{j}_{m}")
                        nc.vector.tensor_add(
                            out=ot[:], in0=ps[:],
                            in1=bias_sb[:, j * NTILE:(j + 1) * NTILE])
                        nc.sync.dma_start(
                            out=out[m * P:(m + 1) * P,
                                    j * NTILE:(j + 1) * NTILE],
                            in_=ot[:])

            if iters == 1:
                body()
            else:
                # Timing variant: repeat the whole kernel on device so the
                # wall-clock slope between two iter counts isolates
                # per-execution device time from dispatch overhead.
                with tc.For_i(0, iters, 1):
                    body()
    nc.compile()
    return nc


def _get_program():
    if "nc" not in _program_cache:
        _program_cache["nc"] = _build_program()
    return _program_cache["nc"]


def _prep_inputs(x, mat, bias):
    """Host-side shard + relayout + bf16 cast.

    xm[m, p, k*128 + b] = x[m*128 + b, k*128 + p]   (shared across cores)
    mj[j, p, k*500 + c] = mat_shard[j*500 + c, k*128 + p]
    """
    x2 = x.reshape(B, E).astype(_BF16)
    xm = np.ascontiguousarray(
        x2.reshape(MT, P, KT, P).transpose(0, 3, 2, 1)
    ).reshape(MT, P, KT * P)
    in_maps = []
    m2 = mat.reshape(C, E)
    b2 = bias.reshape(1, C).astype(np.float32)
    for c in range(NCORES):
        shard = m2[c * CS:(c + 1) * CS].astype(_BF16)  # (CS, E)
        mjc = np.ascontiguousarray(
            shard.reshape(NT, NTILE, KT, P).transpose(0, 3, 2, 1)
        ).reshape(NT, P, KT * NTILE)
        in_maps.append({
            "xm": xm,
            "mj": mjc,
            "bias": np.ascontiguousarray(b2[:, c * CS:(c + 1) * CS]),
        })
    return in_maps


def _run(in_maps, trace=False):
    nc = _get_program()
    return run_bass_kernel_spmd(nc, in_maps, core_ids=list(range(NCORES)),
                                trace=trace)


def _get_runner():
    """jit(shard_map(bass_exec)) built once and cached, so repeat kernel()
    calls skip XLA re-lowering.  No donation: the kernel writes every
    output element, so un-donated result buffers are fine and the staged
    zero buffers can be reused across calls."""
    if "runner" in _program_cache:
        return _program_cache["runner"]
    import jax
    from jax.sharding import Mesh, PartitionSpec, NamedSharding
    from jax.experimental.shard_map import shard_map
    from concourse import bass2jax
    from concourse.bass2jax import _bass_exec_p

    nc = _get_program()
    bass2jax.install_neuronx_cc_hook()
    in_names, out_names, out_avals = [], [], []
    for alloc in nc.m.functions[0].allocations:
        if not isinstance(alloc, mybir.MemoryLocationSet):
            continue
        name = alloc.memorylocations[0].name
        if alloc.kind == "ExternalInput":
            in_names.append(name)
        elif alloc.kind == "ExternalOutput":
            out_names.append(name)
            out_avals.append(jax.core.ShapedArray(
                tuple(alloc.tensor_shape), mybir.dt.np(alloc.dtype)))
    part_name = (nc.partition_id_tensor.name
                 if nc.partition_id_tensor else None)
    if part_name is not None:
        in_names = [n for n in in_names if n != part_name]
    n_params = len(in_names)
    all_names = in_names + out_names
    if part_name is not None:
        all_names = all_names + [part_name]

    def _body(*args):
        operands = list(args)
        if part_name is not None:
            operands.append(bass2jax.partition_id_tensor())
        return tuple(_bass_exec_p.bind(
            *operands,
            out_avals=tuple(out_avals),
            in_names=tuple(all_names),
            out_names=tuple(out_names),
            lowering_input_output_aliases=(),
            sim_require_finite=True,
            sim_require_nnan=True,
            nc=nc,
        ))

    devices = jax.devices()[:NCORES]
    mesh = Mesh(np.asarray(devices), ("core",))
    nspec = (PartitionSpec("core"),) * (n_params + len(out_names))
    sharded = jax.jit(
        shard_map(_body, mesh=mesh, in_specs=nspec,
                  out_specs=(PartitionSpec("core"),) * len(out_names),
                  check_rep=False),
        keep_unused=True)
    sh = NamedSharding(mesh, PartitionSpec("core"))
    zeros = [jax.device_put(
        np.zeros((NCORES * a.shape[0], *a.shape[1:]), a.dtype), sh)
        for a in out_avals]

    def run(in_maps):
        concat_in = [
            jax.device_put(np.concatenate(
                [np.asarray(in_maps[c][name]) for c in range(NCORES)],
                axis=0), sh)
            for name in in_names
        ]
        out = sharded(*concat_in, *zeros)
        jax.block_until_ready(out)
        got = np.asarray(out[out_names.index("out")])
        return got.reshape(NCORES, B, CS)

    _program_cache["runner"] = run
    return run


def kernel(x, mat, bias):
    in_maps = _prep_inputs(np.asarray(x), np.asarray(mat), np.asarray(bias))
    try:
        shards = _get_runner()(in_maps)
        return np.ascontiguousarray(
            shards.transpose(1, 0, 2).reshape(B, C))
    except Exception:
        res = _run(in_maps)
        return np.concatenate(
            [res.results[c]["out"] for c in range(NCORES)], axis=1)


# revision 7
# speedup vs baseline: 1.0717x; 1.0717x over previous
"""Vocab-parallel dense layer (x @ mat^T + bias) on 8 TRN2 NeuronCores.

Full-input contract: kernel(x, mat, bias) takes the unsharded numpy inputs
  x    (4096, 1, 1024) f32
  mat  (1, 32000, 1024) f32
  bias (1, 32000) f32
and returns the full (4096, 32000) f32 output.

Sharding: mat/bias are split over num_classes into 8 shards of 4000 columns
(tensor/vocab parallel); x is replicated.  Each core computes its
(4096, 4000) output slice; the host concatenates.

Device kernel (per core), structured around three HW-measured costs:
per-matmul LDWEIGHTS is NOT hidden (~100-160ns each, so stationary reuse
matters), clustered PSUM evictions stall the PE (~110us at worst), and
the input-load phase serializes ahead of compute unless arrival is
sequenced.
  - Host lays out x as 32 contiguous per-batch-block tiles xm[m] =
    [128p, k*128+b] and mat as 8 contiguous per-class-slice tiles
    mj[j] = [128p, k*500+c], both bf16.
  - Two half-passes over the class dim (n 0-3, then n 4-7).  Within a
    half-pass: m (32 batch tiles) -> k (8 K-tiles) -> n (4 slices): the
    stationary x[m,k] is reused across 4 matmuls (1:4 LDW after dedup)
    and each m uses only 4 PSUM banks, so consecutive m alternate bank
    sets: evictions of m's 4 banks overlap m+1's whole k-pass instead of
    stalling its first matmuls.
  - Input DMAs are sequenced on the two HWDGE FIFO rings in consumption
    order: ACT ring carries mat n0-3 (first half-pass gate, ~4MB), then
    bias, then mat n4-7; SP ring carries x m-blocks (256KB each, so m0
    lands ~1.5us in and blocks trickle ahead of consumption).  Output
    DMAs are emitted on the SP ring behind all x loads, so stores cannot
    steal HBM bandwidth during the load phase; the out-tile ring is deep
    enough for evictions to run ahead while stores wait.
  - PSUM eviction fuses the bias add (vector.tensor_add) and feeds the
    output DMA.
  - Matmuls are chained with ordering-only deps to keep the PE stream in
    emission order, then post-compile passes dedup back-to-back
    LDWEIGHTS with identical weight APs (1536 of 2048 removed).
"""

import numpy as np

import concourse.bass as bass
import concourse.bacc as bacc
import concourse.tile as tile
from concourse import mybir
from concourse.bass import _add_dep_helper
from concourse.bass_utils import run_bass_kernel_spmd

# Problem geometry (hardcoded; harness runs this file standalone).
B = 4096          # batch
E = 1024          # embed size (contraction dim K)
C = 32000         # num classes
NCORES = 8
CS = C // NCORES  # classes per core (4000)

P = 128           # SBUF partitions / matmul tile K and M
KT = E // P       # 8 K-tiles
MT = B // P       # 32 batch tiles
NTILE = 500       # moving free dim per matmul (<=512, one PSUM bank in f32)
NT = CS // NTILE  # 8 class tiles per batch tile
OUTBUF = 24       # SBUF out-tile ring depth (absorbs store lag in load phase)

_BF16 = mybir.dt.np(mybir.dt.bfloat16)

_program_cache = {}


def _dedup_ldweights(nc):
    """Remove InstLdweights whose weights AP matches the immediately
    preceding LDW in the same block's PE stream (the PE array already holds
    those weights).  Only clean LDWs (no waits/updates) are removed."""
    removed = 0
    for fn in nc.m.functions:
        for blk in fn.blocks:
            cur = None
            keep = []
            changed = False
            for inst in blk.instructions:
                if getattr(inst, "engine", None) == mybir.EngineType.PE:
                    if isinstance(inst, mybir.InstLdweights):
                        si = inst.sync_info
                        clean = not (si and (si.on_wait or si.on_update))
                        fp = repr(inst.ins[0])
                        if clean and cur == fp:
                            removed += 1
                            changed = True
                            continue
                        cur = fp
                    elif isinstance(inst, mybir.InstMatmult):
                        pass  # matmul does not disturb loaded weights
                    else:
                        cur = None  # conservative on any other PE inst
                keep.append(inst)
            if changed:
                blk.instructions = keep
    return removed


def _build_program(iters=1):
    bf16, f32 = mybir.dt.bfloat16, mybir.dt.float32

    nc = bacc.Bacc("TRN2", target_bir_lowering=False, debug=False,
                   num_devices=NCORES)
    xm = nc.dram_tensor("xm", (MT, P, KT * P), bf16,
                        kind="ExternalInput").ap()
    mj = nc.dram_tensor("mj", (NT, P, KT * NTILE), bf16,
                        kind="ExternalInput").ap()
    bias = nc.dram_tensor("bias", (1, CS), f32, kind="ExternalInput").ap()
    out = nc.dram_tensor("out", (B, CS), f32, kind="ExternalOutput").ap()

    with tile.TileContext(nc) as tc:
        with tc.tile_pool(name="resident", bufs=1) as resident, \
             tc.tile_pool(name="psum", bufs=8, space="PSUM") as psums, \
             tc.tile_pool(name="outs", bufs=OUTBUF) as outs:

            def body():
                # ACT ring: mat n0-3 (gate of the first half-pass), then
                # the bias broadcast (first needed by the first eviction),
                # then mat n4-7 (needed ~150us in).
                bias_sb = resident.tile([P, CS], f32, tag="bias",
                                        name="bias_sb")
                msb = []
                for j in range(NT):
                    mt = resident.tile([P, KT * NTILE], bf16, tag=f"m{j}",
                                       name=f"m{j}")
                    msb.append(mt)
                for j in (0, 1, 2, 3):
                    nc.scalar.dma_start(out=msb[j][:], in_=mj[j])
                nc.scalar.dma_start(out=bias_sb[:],
                                    in_=bias.to_broadcast((P, CS)))
                for j in (4, 5, 6, 7):
                    nc.scalar.dma_start(out=msb[j][:], in_=mj[j])
                # SP ring: x blocks in consumption order; output stores are
                # emitted later on this same FIFO ring so they queue behind
                # every x load.
                xsb = []
                for m in range(MT):
                    xt = resident.tile([P, KT * P], bf16, tag=f"x{m}",
                                       name=f"x{m}")
                    nc.sync.dma_start(out=xt[:], in_=xm[m])
                    xsb.append(xt)

                prev_mm = None
                HN = NT // 2  # slices per half-pass (4 PSUM banks per m)
                for half in range(2):
                    for m in range(MT):
                        pss = [psums.tile([P, NTILE], f32, tag="ps",
                                          name=f"ps{half}_{m}_{n}")
                               for n in range(HN)]
                        for k in range(KT):
                            for n in range(HN):
                                j = half * HN + n
                                mm = nc.tensor.matmul(
                                    pss[n][:],
                                    xsb[m][:, k * P:(k + 1) * P],
                                    msb[j][:,
                                           k * NTILE:(k + 1) * NTILE],
                                    start=(k == 0), stop=(k == KT - 1))
                                # ordering-only chain keeps the PE stream
                                # in emission order
                                if prev_mm is not None:
                                    _add_dep_helper(mm.ins, prev_mm.ins,
                                                    sync=False,
                                                    reason="pe-order")
                                prev_mm = mm
                        for n in range(HN):
                            j = half * HN + n
                            ot = outs.tile([P, NTILE], f32, tag="ot",
                                           name=f"ot{half}_{m}_{n}")
                            nc.vector.tensor_add(
                                out=ot[:], in0=pss[n][:],
                                in1=bias_sb[:,
                                            j * NTILE:(j + 1) * NTILE])
                            nc.sync.dma_start(
                                out=out[m * P:(m + 1) * P,
                                        j * NTILE:(j + 1) * NTILE],
                                in_=ot[:])

            if iters == 1:
                body()
            else:
                # Timing variant: repeat the whole kernel on device so the
                # wall-clock slope between two iter counts isolates
                # per-execution device time from dispatch overhead.
                with tc.For_i(0, iters, 1):
                    body()
    nc.compile()
    _dedup_ldweights(nc)
    return nc


def _get_program():
    if "nc" not in _program_cache:
        _program_cache["nc"] = _build_program()
    return _program_cache["nc"]


def _prep_inputs(x, mat, bias):
    """Host-side shard + relayout + bf16 cast.

    xm[m, p, k*128 + b] = x[m*128 + b, k*128 + p]   (shared across cores)
    mj[j, p, k*500 + c] = mat_shard[j*500 + c, k*128 + p]
    """
    x2 = x.reshape(B, E).astype(_BF16)
    xm = np.ascontiguousarray(
        x2.reshape(MT, P, KT, P).transpose(0, 3, 2, 1)
    ).reshape(MT, P, KT * P)
    in_maps = []
    m2 = mat.reshape(C, E)
    b2 = bias.reshape(1, C).astype(np.float32)
    for c in range(NCORES):
        shard = m2[c * CS:(c + 1) * CS].astype(_BF16)  # (CS, E)
        mjc = np.ascontiguousarray(
            shard.reshape(NT, NTILE, KT, P).transpose(0, 3, 2, 1)
        ).reshape(NT, P, KT * NTILE)
        in_maps.append({
            "xm": xm,
            "mj": mjc,
            "bias": np.ascontiguousarray(b2[:, c * CS:(c + 1) * CS]),
        })
    return in_maps


def _run(in_maps, trace=False):
    nc = _get_program()
    return run_bass_kernel_spmd(nc, in_maps, core_ids=list(range(NCORES)),
                                trace=trace)


def _get_runner():
    """jit(shard_map(bass_exec)) built once and cached, so repeat kernel()
    calls skip XLA re-lowering.  No donation: the kernel writes every
    output element, so un-donated result buffers are fine and the staged
    zero buffers can be reused across calls."""
    if "runner" in _program_cache:
        return _program_cache["runner"]
    import jax
    from jax.sharding import Mesh, PartitionSpec, NamedSharding
    from jax.experimental.shard_map import shard_map
    from concourse import bass2jax
    from concourse.bass2jax import _bass_exec_p

    nc = _get_program()
    bass2jax.install_neuronx_cc_hook()
    in_names, out_names, out_avals = [], [], []
    for alloc in nc.m.functions[0].allocations:
        if not isinstance(alloc, mybir.MemoryLocationSet):
            continue
        name = alloc.memorylocations[0].name
        if alloc.kind == "ExternalInput":
            in_names.append(name)
        elif alloc.kind == "ExternalOutput":
            out_names.append(name)
            out_avals.append(jax.core.ShapedArray(
                tuple(alloc.tensor_shape), mybir.dt.np(alloc.dtype)))
    part_name = (nc.partition_id_tensor.name
                 if nc.partition_id_tensor else None)
    if part_name is not None:
        in_names = [n for n in in_names if n != part_name]
    n_params = len(in_names)
    all_names = in_names + out_names
    if part_name is not None:
        all_names = all_names + [part_name]

    def _body(*args):
        operands = list(args)
        if part_name is not None:
            operands.append(bass2jax.partition_id_tensor())
        return tuple(_bass_exec_p.bind(
            *operands,
            out_avals=tuple(out_avals),
            in_names=tuple(all_names),
            out_names=tuple(out_names),
            lowering_input_output_aliases=(),
            sim_require_finite=True,
            sim_require_nnan=True,
            nc=nc,
        ))

    devices = jax.devices()[:NCORES]
    mesh = Mesh(np.asarray(devices), ("core",))
    nspec = (PartitionSpec("core"),) * (n_params + len(out_names))
    sharded = jax.jit(
        shard_map(_body, mesh=mesh, in_specs=nspec,
                  out_specs=(PartitionSpec("core"),) * len(out_names),
                  check_rep=False),
        keep_unused=True)
    sh = NamedSharding(mesh, PartitionSpec("core"))
    zeros = [jax.device_put(
        np.zeros((NCORES * a.shape[0], *a.shape[1:]), a.dtype), sh)
        for a in out_avals]

    def run(in_maps):
        concat_in = [
            jax.device_put(np.concatenate(
                [np.asarray(in_maps[c][name]) for c in range(NCORES)],
                axis=0), sh)
            for name in in_names
        ]
        out = sharded(*concat_in, *zeros)
        jax.block_until_ready(out)
        got = np.asarray(out[out_names.index("out")])
        return got.reshape(NCORES, B, CS)

    _program_cache["runner"] = run
    return run


def kernel(x, mat, bias):
    in_maps = _prep_inputs(np.asarray(x), np.asarray(mat), np.asarray(bias))
    try:
        shards = _get_runner()(in_maps)
        return np.ascontiguousarray(
            shards.transpose(1, 0, 2).reshape(B, C))
    except Exception:
        res = _run(in_maps)
        return np.concatenate(
            [res.results[c]["out"] for c in range(NCORES)], axis=1)


# revision 13
# speedup vs baseline: 1.0741x; 1.0023x over previous
"""Vocab-parallel dense layer (x @ mat^T + bias) on 8 TRN2 NeuronCores.

Full-input contract: kernel(x, mat, bias) takes the unsharded numpy inputs
  x    (4096, 1, 1024) f32
  mat  (1, 32000, 1024) f32
  bias (1, 32000) f32
and returns the full (4096, 32000) f32 output.

Sharding: mat/bias are split over num_classes into 8 shards of 4000 columns
(tensor/vocab parallel); x is replicated.  Each core computes its
(4096, 4000) output slice; the host concatenates.

Device kernel (per core).  HW-measured findings this structure encodes
(same-process interleaved slope A/B; cross-process slope numbers are noisy
by +-150us):
  - Any bf16 kernel streams 1.024M moving columns through the PE at 1
    col/cycle @2.4GHz -> ~427us floor; DMA (19MB in + 65.5MB out at
    358GB/s) hides under it.  fp8 cannot pass the accuracy gate.
  - Clustered PSUM evictions (all 8 banks completing together per batch
    tile, as in the m->k->n baseline) cost ~+100us of PE stalls; the
    fine-grained structure below measured best (507us vs 634us baseline).
  - LDWEIGHTS count matters far less than expected on this silicon.
Structure:
  - Host lays out x as 32 contiguous per-batch-block tiles xm[m] =
    [128p, k*128+b] and mat as 8 contiguous per-class-slice tiles
    mj[j] = [128p, k*500+c], both bf16.
  - Loop j (8 class slices) -> m (32 batch tiles) -> k (8 K-tiles): each
    (j,m) is one 8-matmul accumulation k-run into a single PSUM bank,
    banks rotating 8-deep, so each bank's eviction overlaps the next 7
    k-runs (~13us of slack) instead of clustering.
  - Input DMAs are sequenced on the two HWDGE FIFO rings in consumption
    order.  ACT ring: bias slice j0 (lands ~0.6us, unblocks the first
    eviction), mat j0 (gates the first matmul, ~3us), rest of bias, mat
    j1, then mat j2..j7 each dep-gated on the first matmul of slice j-2
    so the ACT sequencer stalls and x keeps full HBM read bandwidth.
    SP ring: x m-blocks (256KB each; m0 lands ~1.5us in, arrival rate
    0.73us/block stays ahead of the 1.9us/k-run consumption).  Output
    DMAs are emitted on the SP ring behind all x loads, so stores cannot
    steal read bandwidth during the load phase; the out-tile ring is 24
    deep so evictions run ahead while stores queue.
  - PSUM eviction fuses the bias add (vector.tensor_add) and feeds the
    output DMA.
  - Matmuls are chained with ordering-only deps to keep the PE stream in
    emission order; a post-compile pass dedups back-to-back LDWEIGHTS
    with identical weight APs (no-op for this structure, kept for the
    halfbank variant).
"""

import numpy as np

import concourse.bass as bass
import concourse.bacc as bacc
import concourse.tile as tile
from concourse import mybir
from concourse.bass import _add_dep_helper
from concourse.bass_utils import run_bass_kernel_spmd

# Problem geometry (hardcoded; harness runs this file standalone).
B = 4096          # batch
E = 1024          # embed size (contraction dim K)
C = 32000         # num classes
NCORES = 8
CS = C // NCORES  # classes per core (4000)

P = 128           # SBUF partitions / matmul tile K and M
KT = E // P       # 8 K-tiles
MT = B // P       # 32 batch tiles
NTILE = 500       # moving free dim per matmul (<=512, one PSUM bank in f32)
NT = CS // NTILE  # 8 class tiles per batch tile
OUTBUF = 24       # SBUF out-tile ring depth (absorbs store lag in load phase)

_BF16 = mybir.dt.np(mybir.dt.bfloat16)

_program_cache = {}


def _dedup_ldweights(nc):
    """Remove InstLdweights whose weights AP matches the immediately
    preceding LDW in the same block's PE stream (the PE array already holds
    those weights).  Only clean LDWs (no waits/updates) are removed."""
    removed = 0
    for fn in nc.m.functions:
        for blk in fn.blocks:
            cur = None
            keep = []
            changed = False
            for inst in blk.instructions:
                if getattr(inst, "engine", None) == mybir.EngineType.PE:
                    if isinstance(inst, mybir.InstLdweights):
                        si = inst.sync_info
                        clean = not (si and (si.on_wait or si.on_update))
                        fp = repr(inst.ins[0])
                        if clean and cur == fp:
                            removed += 1
                            changed = True
                            continue
                        cur = fp
                    elif isinstance(inst, mybir.InstMatmult):
                        pass  # matmul does not disturb loaded weights
                    else:
                        cur = None  # conservative on any other PE inst
                keep.append(inst)
            if changed:
                blk.instructions = keep
    return removed


def _thin_mm_updates(nc):
    """Re-key the PE matmul semaphore from per-MM counts to per-stop-MM
    counts: strip the +1 update from non-stop MMs (walrus requires
    UpdateValue==1) and remap every sem-ge-imm wait on that sem — old
    threshold V (satisfied by MM #V) becomes the index of the first
    stop-MM at or after MM #V.  PE executes in order, so the remapped
    wait fires iff at least the same MMs have retired — never earlier."""
    import bisect
    target_blk, mms = None, None
    for fn in nc.m.functions:
        for blk in fn.blocks:
            cand = [i for i in blk.instructions
                    if isinstance(i, mybir.InstMatmult)]
            if len(cand) >= 16:
                assert target_blk is None, "expected a single MM block"
                target_blk, mms = blk, cand
    if mms is None:
        return 0
    sem_ids = set()
    for mm in mms:
        si = mm.sync_info
        for u in (si.on_update if si else []):
            if u.sync_type == "semaphore" and u.update_value == 1:
                sem_ids.add(u.id)
    if len(sem_ids) != 1:
        return 0
    sem = sem_ids.pop()
    stop_old = [i + 1 for i, mm in enumerate(mms) if mm.stop_tensor_calc]

    def remap(v):
        if v <= 0:
            return v
        j = bisect.bisect_left(stop_old, v)
        assert j < len(stop_old), f"wait {v} beyond final MM count"
        return j + 1

    for fn in nc.m.functions:
        for blk in fn.blocks:
            for inst in blk.instructions:
                si = inst.sync_info
                for w in (si.on_wait if si else []):
                    if (w.sync_type == "semaphore" and w.id == sem
                            and w.wait_value is not None):
                        assert w.wait_mode == "sem-ge-imm", w.wait_mode
                        assert w.wait_reg is None
                        w.wait_value = remap(w.wait_value)
    removed = 0
    for mm in mms:
        si = mm.sync_info
        ups = si.on_update if si else []
        mine = [u for u in ups
                if u.sync_type == "semaphore" and u.id == sem]
        if mine and not mm.stop_tensor_calc:
            ups.remove(mine[0])
            removed += 1
    return removed


def _build_program(iters=1, structure="permm", thin=False, chain=True):
    bf16, f32 = mybir.dt.bfloat16, mybir.dt.float32

    nc = bacc.Bacc("TRN2", target_bir_lowering=False, debug=False,
                   num_devices=NCORES)
    xm = nc.dram_tensor("xm", (MT, P, KT * P), bf16,
                        kind="ExternalInput").ap()
    mj = nc.dram_tensor("mj", (NT, P, KT * NTILE), bf16,
                        kind="ExternalInput").ap()
    bias = nc.dram_tensor("bias", (1, CS), f32, kind="ExternalInput").ap()
    out = nc.dram_tensor("out", (B, CS), f32, kind="ExternalOutput").ap()

    with tile.TileContext(nc) as tc:
        with tc.tile_pool(name="resident", bufs=1) as resident, \
             tc.tile_pool(name="psum", bufs=8, space="PSUM") as psums, \
             tc.tile_pool(name="outs", bufs=OUTBUF) as outs:

            def body():
                # ACT ring (FIFO): bias slice j0 (0.25MB write, lands
                # first so the first eviction never waits), mat j0 (gates
                # the first matmul, ~3us), the rest of the bias, mat j1,
                # then mat j2..j7 each dep-gated on the first matmul of
                # slice j-2 so the ACT sequencer stalls and the SP ring's
                # x loads get full HBM read bandwidth during the j0 pass.
                bias_sb = resident.tile([P, CS], f32, tag="bias",
                                        name="bias_sb")
                msb = []
                for j in range(NT):
                    mt = resident.tile([P, KT * NTILE], bf16, tag=f"m{j}",
                                       name=f"m{j}")
                    msb.append(mt)
                nc.scalar.dma_start(
                    out=bias_sb[:, 0:NTILE],
                    in_=bias[:, 0:NTILE].to_broadcast((P, NTILE)))
                mat_dmas = [nc.scalar.dma_start(out=msb[0][:], in_=mj[0])]
                nc.scalar.dma_start(
                    out=bias_sb[:, NTILE:CS],
                    in_=bias[:, NTILE:CS].to_broadcast((P, CS - NTILE)))
                for j in range(1, NT):
                    mat_dmas.append(
                        nc.scalar.dma_start(out=msb[j][:], in_=mj[j]))
                # SP ring: x blocks in consumption order; output stores are
                # emitted later on this same FIFO ring so they queue behind
                # every x load.
                xsb = []
                for m in range(MT):
                    xt = resident.tile([P, KT * P], bf16, tag=f"x{m}",
                                       name=f"x{m}")
                    nc.sync.dma_start(out=xt[:], in_=xm[m])
                    xsb.append(xt)

                prev_mm = None
                first_mm = {}
                if structure == "permm":
                    loop = [(j, m, (j, m)) for j in range(NT)
                            for m in range(MT)]
                    for j, m, key in loop:
                        ps = psums.tile([P, NTILE], f32, tag="ps",
                                        name=f"ps{j}_{m}")
                        for k in range(KT):
                            mm = nc.tensor.matmul(
                                ps[:],
                                xsb[m][:, k * P:(k + 1) * P],
                                msb[j][:, k * NTILE:(k + 1) * NTILE],
                                start=(k == 0), stop=(k == KT - 1))
                            if chain and prev_mm is not None:
                                _add_dep_helper(mm.ins, prev_mm.ins,
                                                sync=False,
                                                reason="pe-order")
                            prev_mm = mm
                            if j not in first_mm:
                                first_mm[j] = mm
                        ot = outs.tile([P, NTILE], f32, tag="ot",
                                       name=f"ot{j}_{m}")
                        nc.vector.tensor_add(
                            out=ot[:], in0=ps[:],
                            in1=bias_sb[:, j * NTILE:(j + 1) * NTILE])
                        nc.sync.dma_start(
                            out=out[m * P:(m + 1) * P,
                                    j * NTILE:(j + 1) * NTILE],
                            in_=ot[:])
                else:  # halfbank
                    HN = NT // 2
                    for half in range(2):
                        for m in range(MT):
                            pss = [psums.tile([P, NTILE], f32, tag="ps",
                                              name=f"ps{half}_{m}_{n}")
                                   for n in range(HN)]
                            for k in range(KT):
                                for n in range(HN):
                                    j = half * HN + n
                                    mm = nc.tensor.matmul(
                                        pss[n][:],
                                        xsb[m][:, k * P:(k + 1) * P],
                                        msb[j][:,
                                               k * NTILE:(k + 1) * NTILE],
                                        start=(k == 0),
                                        stop=(k == KT - 1))
                                    if prev_mm is not None:
                                        _add_dep_helper(mm.ins,
                                                        prev_mm.ins,
                                                        sync=False,
                                                        reason="pe-order")
                                    prev_mm = mm
                                    if j not in first_mm:
                                        first_mm[j] = mm
                            for n in range(HN):
                                j = half * HN + n
                                ot = outs.tile([P, NTILE], f32, tag="ot",
                                               name=f"ot{half}_{m}_{n}")
                                nc.vector.tensor_add(
                                    out=ot[:], in0=pss[n][:],
                                    in1=bias_sb[:,
                                                j * NTILE:
                                                (j + 1) * NTILE])
                                nc.sync.dma_start(
                                    out=out[m * P:(m + 1) * P,
                                            j * NTILE:(j + 1) * NTILE],
                                    in_=ot[:])
                # Gate mat slice j>=2 on the first matmul of slice j-2:
                # the ACT sequencer stalls there, keeping HBM read
                # bandwidth on the x loads until each slice is nearly
                # needed.
                for j in range(2, NT):
                    gate = first_mm.get(j - 2)
                    if gate is not None:
                        _add_dep_helper(mat_dmas[j].ins, gate.ins,
                                        sync=True, reason="mat-gate")

            if iters == 1:
                body()
            else:
                # Timing variant: repeat the whole kernel on device so the
                # wall-clock slope between two iter counts isolates
                # per-execution device time from dispatch overhead.
                with tc.For_i(0, iters, 1):
                    body()
    nc.compile()
    _dedup_ldweights(nc)
    if thin:
        _thin_mm_updates(nc)
    return nc


def _get_program():
    if "nc" not in _program_cache:
        _program_cache["nc"] = _build_program()
    return _program_cache["nc"]


def _prep_inputs(x, mat, bias):
    """Host-side shard + relayout + bf16 cast.

    xm[m, p, k*128 + b] = x[m*128 + b, k*128 + p]   (shared across cores)
    mj[j, p, k*500 + c] = mat_shard[j*500 + c, k*128 + p]
    """
    x2 = x.reshape(B, E).astype(_BF16)
    xm = np.ascontiguousarray(
        x2.reshape(MT, P, KT, P).transpose(0, 3, 2, 1)
    ).reshape(MT, P, KT * P)
    in_maps = []
    m2 = mat.reshape(C, E)
    b2 = bias.reshape(1, C).astype(np.float32)
    for c in range(NCORES):
        shard = m2[c * CS:(c + 1) * CS].astype(_BF16)  # (CS, E)
        mjc = np.ascontiguousarray(
            shard.reshape(NT, NTILE, KT, P).transpose(0, 3, 2, 1)
        ).reshape(NT, P, KT * NTILE)
        in_maps.append({
            "xm": xm,
            "mj": mjc,
            "bias": np.ascontiguousarray(b2[:, c * CS:(c + 1) * CS]),
        })
    return in_maps


def _run(in_maps, trace=False):
    nc = _get_program()
    return run_bass_kernel_spmd(nc, in_maps, core_ids=list(range(NCORES)),
                                trace=trace)


def _get_runner():
    """jit(shard_map(bass_exec)) built once and cached, so repeat kernel()
    calls skip XLA re-lowering.  No donation: the kernel writes every
    output element, so un-donated result buffers are fine and the staged
    zero buffers can be reused across calls."""
    if "runner" in _program_cache:
        return _program_cache["runner"]
    import jax
    from jax.sharding import Mesh, PartitionSpec, NamedSharding
    from jax.experimental.shard_map import shard_map
    from concourse import bass2jax
    from concourse.bass2jax import _bass_exec_p

    nc = _get_program()
    bass2jax.install_neuronx_cc_hook()
    in_names, out_names, out_avals = [], [], []
    for alloc in nc.m.functions[0].allocations:
        if not isinstance(alloc, mybir.MemoryLocationSet):
            continue
        name = alloc.memorylocations[0].name
        if alloc.kind == "ExternalInput":
            in_names.append(name)
        elif alloc.kind == "ExternalOutput":
            out_names.append(name)
            out_avals.append(jax.core.ShapedArray(
                tuple(alloc.tensor_shape), mybir.dt.np(alloc.dtype)))
    part_name = (nc.partition_id_tensor.name
                 if nc.partition_id_tensor else None)
    if part_name is not None:
        in_names = [n for n in in_names if n != part_name]
    n_params = len(in_names)
    all_names = in_names + out_names
    if part_name is not None:
        all_names = all_names + [part_name]

    def _body(*args):
        operands = list(args)
        if part_name is not None:
            operands.append(bass2jax.partition_id_tensor())
        return tuple(_bass_exec_p.bind(
            *operands,
            out_avals=tuple(out_avals),
            in_names=tuple(all_names),
            out_names=tuple(out_names),
            lowering_input_output_aliases=(),
            sim_require_finite=True,
            sim_require_nnan=True,
            nc=nc,
        ))

    devices = jax.devices()[:NCORES]
    mesh = Mesh(np.asarray(devices), ("core",))
    nspec = (PartitionSpec("core"),) * (n_params + len(out_names))
    sharded = jax.jit(
        shard_map(_body, mesh=mesh, in_specs=nspec,
                  out_specs=(PartitionSpec("core"),) * len(out_names),
                  check_rep=False),
        keep_unused=True)
    sh = NamedSharding(mesh, PartitionSpec("core"))
    zeros = [jax.device_put(
        np.zeros((NCORES * a.shape[0], *a.shape[1:]), a.dtype), sh)
        for a in out_avals]

    def run(in_maps):
        concat_in = [
            jax.device_put(np.concatenate(
                [np.asarray(in_maps[c][name]) for c in range(NCORES)],
                axis=0), sh)
            for name in in_names
        ]
        out = sharded(*concat_in, *zeros)
        jax.block_until_ready(out)
        got = np.asarray(out[out_names.index("out")])
        return got.reshape(NCORES, B, CS)

    _program_cache["runner"] = run
    return run


def kernel(x, mat, bias):
    in_maps = _prep_inputs(np.asarray(x), np.asarray(mat), np.asarray(bias))
    try:
        shards = _get_runner()(in_maps)
        return np.ascontiguousarray(
            shards.transpose(1, 0, 2).reshape(B, C))
    except Exception:
        res = _run(in_maps)
        return np.concatenate(
            [res.results[c]["out"] for c in range(NCORES)], axis=1)
